# revision 20
# baseline (speedup 1.0000x reference)
"""GIN-style 5-layer GNN encoder on 8 TRN2 NeuronCores (Bass/Tile SPMD).

Sharding: nodes split contiguously across 8 cores (6250 each); edges
partitioned by destination core. Per layer, each core gathers the source-node
rows of its incident edges from a replicated row-major copy of h in HBM
(dma_gather), reduces them into 128-node destination windows with 0/1
selection matmuls on the TensorEngine (agg_T[f,slot] += X[e,f].T @ S[e,slot]),
runs the MLP + BatchNorm feat-major, transposes back to node rows on the PE,
and republishes its h shard via a two-phase AllGather (shard halves) that
overlaps the next layer's gather phase. BatchNorm statistics use one small
AllReduce per layer. Graph pooling is a per-window Ind-matmul into a per-core
graph window; per-core partials are combined on the host (unshard step).
"""

import sys

import numpy as np

sys.path.insert(0, "/opt/trn_rl_repo")
sys.path.insert(0, "/opt/trn_rl_repo/concourse")

import concourse.bass as bass
import concourse.mybir as mybir
from concourse import bacc, tile

FP32 = mybir.dt.float32
I16 = mybir.dt.int16
AX = mybir.AluOpType
AF = mybir.ActivationFunctionType
XAX = mybir.AxisListType.X

EPS_GIN = 0.1
BN_EPS = 1e-5
CH = 512  # feat-major chunk width for elementwise passes


def full_cfg():
    return dict(N=50000, E=800000, G=256, D=128, L=5, C=8)


def derive(cfg):
    N, C = cfg["N"], cfg["C"]
    NL = N // C
    assert NL * C == N
    HALF = (NL + 1) // 2
    W = (NL + 127) // 128
    NG = (W + 1) // 2
    return dict(NL=NL, HALF=HALF, HB=NL - HALF, W=W, NG=NG)


# ---------------------------------------------------------------------------
# Host preprocessing (index/layout only + input-vector normalization)
# ---------------------------------------------------------------------------

def preprocess(cfg, x, node_imp, edge_index, batch):
    N, E, G, D, L, C = (cfg[k] for k in ("N", "E", "G", "D", "L", "C"))
    dv = derive(cfg)
    NL, HALF, HB, W, NG = dv["NL"], dv["HALF"], dv["HB"], dv["W"], dv["NG"]

    x = np.ascontiguousarray(np.asarray(x, np.float32))
    imp_raw = np.asarray(node_imp, np.float32)[:, 0]
    src = np.asarray(edge_index[0], np.int64)
    dst = np.asarray(edge_index[1], np.int64)
    batch = np.asarray(batch, np.int64)

    gmax = np.full(G, -np.inf, np.float32)
    np.maximum.at(gmax, batch, imp_raw)
    gmax = np.where(np.isfinite(gmax), gmax, np.float32(1.0)).astype(np.float32)
    imp = (imp_raw / (gmax[batch] * 10.0) + 0.9).astype(np.float32)

    owner = src // NL
    pos = src % NL
    in_a = pos < HALF
    srow = np.where(in_a, owner * HALF + pos,
                    owner * HB + (pos - HALF)).astype(np.int64)
    ehalf = (~in_a).astype(np.int64)

    ecore = dst // NL
    ldst = dst % NL
    ewin = ldst // 128
    eslot = ldst % 128

    per_core = []
    for c in range(C):
        m = ecore == c
        order = np.lexsort((srow[m], ewin[m], ehalf[m]))
        per_core.append((ehalf[m][order], ewin[m][order],
                         srow[m][order], eslot[m][order]))

    # tiles per (pass, window), equal across cores
    Tcnt = np.zeros((2, W), np.int64)
    for c in range(C):
        h_, w_, _, _ = per_core[c]
        for p in range(2):
            for w in range(W):
                n = int(np.sum((h_ == p) & (w_ == w)))
                Tcnt[p, w] = max(Tcnt[p, w], (n + 127) // 128)
    Tcnt[0] = np.maximum(Tcnt[0], 1)  # every window gets >=1 pass-0 tile
    TOT_TILES = int(Tcnt.sum())
    TOT_IDX = TOT_TILES * 128

    idx_flat = np.zeros((C, TOT_IDX), np.int16)
    slot_np = np.full((C, 128, TOT_TILES), 255.0, np.float32)
    for c in range(C):
        h_, w_, r_, s_ = per_core[c]
        off = 0
        for p in range(2):
            for w in range(W):
                T = int(Tcnt[p, w])
                if T == 0:
                    continue
                sel = (h_ == p) & (w_ == w)
                rows, slots = r_[sel], s_[sel]
                n = len(rows)
                ii = np.zeros(T * 128, np.int64)
                ii[:n] = rows
                ss = np.full(T * 128, 255.0, np.float32)
                ss[:n] = slots
                idx_flat[c, off * 128:(off + T) * 128] = ii.astype(np.int16)
                slot_np[c, :, off:off + T] = ss.reshape(T, 128).T
                off += T
        assert off == TOT_TILES

    idx_wrap = np.zeros((C, 128, TOT_IDX // 16), np.int16)
    for c in range(C):
        idx_wrap[c] = np.tile(idx_flat[c].reshape(-1, 16).T, (8, 1))

    xa = np.zeros((C * HALF, D), np.float32)
    xb = np.zeros((C * HB, D), np.float32)
    for c in range(C):
        xs_ = x[c * NL:(c + 1) * NL]
        xa[c * HALF:(c + 1) * HALF] = xs_[:HALF]
        xb[c * HB:(c + 1) * HB] = xs_[HALF:]

    NPAD = W * 128
    xT = np.zeros((C, D, NPAD), np.float32)
    imp_loc = np.zeros((C, 128, W), np.float32)
    for c in range(C):
        xT[c, :, :NL] = x[c * NL:(c + 1) * NL].T
        v = np.zeros(NPAD, np.float32)
        v[:NL] = imp[c * NL:(c + 1) * NL]
        imp_loc[c] = v.reshape(W, 128).T

    GW = min(64, G)
    gw_start = np.zeros(C, np.int64)
    ind_np = np.zeros((C, 128, W * GW), np.float32)
    for c in range(C):
        gb = batch[c * NL:(c + 1) * NL]
        g0 = int(gb.min())
        gw_start[c] = g0
        assert int(gb.max()) - g0 < GW, "graph window overflow"
        cols = (gb - g0).astype(np.int64)
        for n in range(NL):
            ind_np[c, n % 128, (n // 128) * GW + cols[n]] = 1.0

    meta = dict(Tcnt=Tcnt, TOT_TILES=TOT_TILES, TOT_IDX=TOT_IDX, GW=GW,
                gw_start=gw_start, NPAD=NPAD)
    iota = np.tile(np.arange(128, dtype=np.float32), (128, 1))
    ioslot = np.concatenate(
        [np.broadcast_to(iota[None], (C, 128, 128)), slot_np], axis=2)
    per_core_inputs = dict(xT=xT, idx=idx_wrap, ioslot=ioslot, imp=imp_loc,
                           ind=ind_np)
    shared_inputs = dict(
        xa=xa, xb=xb,
        ident=np.eye(128, dtype=np.float32),
    )
    return meta, shared_inputs, per_core_inputs


def pack_weights(cfg, W1, b1, W2, b2, gamma, beta):
    D, L = cfg["D"], cfg["L"]
    w1s = np.zeros((D, L * D), np.float32)
    w2s = np.zeros((D, L * D), np.float32)
    for l in range(L):
        w1s[:, l * D:(l + 1) * D] = W1[l]
        w2s[:, l * D:(l + 1) * D] = W2[l]
    return dict(
        w1s=w1s, w2s=w2s,
        b1s=np.asarray(b1, np.float32).T.copy(),
        b2s=np.asarray(b2, np.float32).T.copy(),
        gammas=np.asarray(gamma, np.float32).T.copy(),
        betas=np.asarray(beta, np.float32).T.copy(),
    )


# ---------------------------------------------------------------------------
# Device program (identical across cores)
# ---------------------------------------------------------------------------

def build_program(cfg, meta):
    N, E, G, D, L, C = (cfg[k] for k in ("N", "E", "G", "D", "L", "C"))
    dv = derive(cfg)
    NL, HALF, HB, W, NG = dv["NL"], dv["HALF"], dv["HB"], dv["W"], dv["NG"]
    Tcnt, TOT_TILES, TOT_IDX = meta["Tcnt"], meta["TOT_TILES"], meta["TOT_IDX"]
    GW, NPAD = meta["GW"], meta["NPAD"]
    RG = [list(range(C))]
    LAST = NL - (W - 1) * 128       # valid rows in the last window
    WB = HALF // 128                # window straddling the A/B shard boundary
    SB = HALF % 128

    nc = bacc.Bacc("TRN2", target_bir_lowering=False, debug=False,
                   num_devices=C)

    def param(name, shape, dtype=FP32, out=False):
        return nc.declare_dram_parameter(name, list(shape), dtype,
                                         isOutput=out)

    xa_p = param("xa", [C * HALF, D])
    xb_p = param("xb", [C * HB, D])
    xT_p = param("xT", [D, NPAD])
    idx_p = param("idx", [128, TOT_IDX // 16], I16)
    iosl_p = param("ioslot", [128, 128 + TOT_TILES])
    imp_p = param("imp", [128, W])
    ind_p = param("ind", [128, W * GW])
    ident_p = param("ident", [128, 128])
    w1_p = param("w1s", [D, L * D])
    w2_p = param("w2s", [D, L * D])
    b1_p = param("b1s", [D, L])
    b2_p = param("b2s", [D, L])
    ga_p = param("gammas", [D, L])
    be_p = param("betas", [D, L])

    xcat_p = param("xcat", [NL, L * D], out=True)
    pool_p = param("pooled", [GW, L * D], out=True)

    MAXTG = max(int(Tcnt[p, 2 * g] + (Tcnt[p, 2 * g + 1] if 2 * g + 1 < W
                                      else 0))
                for p in range(2) for g in range(NG))

    from contextlib import ExitStack

    with tile.TileContext(nc) as tc, ExitStack() as ctx:
        const_pool = ctx.enter_context(tc.tile_pool(name="consts", bufs=1))
        big_pool = ctx.enter_context(tc.tile_pool(name="big", bufs=1))
        xbuf_pool = ctx.enter_context(tc.tile_pool(name="xbuf", bufs=2))
        s_pool = ctx.enter_context(tc.tile_pool(name="spool", bufs=4))
        small_pool = ctx.enter_context(tc.tile_pool(name="small", bufs=1))
        mlp_pool = ctx.enter_context(tc.tile_pool(name="mlp", bufs=2))
        scr_pool = ctx.enter_context(tc.tile_pool(name="scr", bufs=2))
        out_pool = ctx.enter_context(tc.tile_pool(name="outp", bufs=3))
        ps_agg = ctx.enter_context(
            tc.tile_pool(name="ps_agg", bufs=3, space="PSUM"))
        ps_mlp = ctx.enter_context(
            tc.tile_pool(name="ps_mlp", bufs=2, space="PSUM"))
        ps_tr = ctx.enter_context(
            tc.tile_pool(name="ps_tr", bufs=2, space="PSUM"))
        ps_pool = ctx.enter_context(
            tc.tile_pool(name="ps_pool", bufs=1, space="PSUM"))
        dram_pool = ctx.enter_context(
            tc.tile_pool(name="dram", bufs=2, space="DRAM"))

        # persistent SBUF
        ident_sb = const_pool.tile([128, 128], FP32)
        idx_sb = const_pool.tile([128, TOT_IDX // 16], I16)
        iosl_sb = const_pool.tile([128, 128 + TOT_TILES], FP32)
        imp_sb = const_pool.tile([128, W], FP32)
        ind_sb = const_pool.tile([128, W * GW], FP32)
        w1_sb = const_pool.tile([D, L * D], FP32)
        w2_sb = const_pool.tile([D, L * D], FP32)
        b1_sb = const_pool.tile([D, L], FP32)
        b2_sb = const_pool.tile([D, L], FP32)
        ga_sb = const_pool.tile([D, L], FP32)
        be_sb = const_pool.tile([D, L], FP32)

        h_T = big_pool.tile([D, NPAD], FP32)
        h11_T = big_pool.tile([D, NPAD], FP32)
        z_T = big_pool.tile([D, NPAD], FP32)
        hpre_T = big_pool.tile([D, NPAD], FP32)
        pooled_sb = big_pool.tile([GW, L * D], FP32)
        sumc_sb = small_pool.tile([128, NG], FP32)
        sqc_sb = small_pool.tile([128, NG], FP32)
        stats_sb = small_pool.tile([128, 2], FP32)
        bn_sb = small_pool.tile([128, 2], FP32)
        mu_sb = small_pool.tile([128, 1], FP32)
        ex2_sb = small_pool.tile([128, 1], FP32)
        var_sb = small_pool.tile([128, 1], FP32)
        std_sb = small_pool.tile([128, 1], FP32)
        rstd_sb = small_pool.tile([128, 1], FP32)
        scale_sb = small_pool.tile([128, 1], FP32)
        shift_sb = small_pool.tile([128, 1], FP32)
        tmp_sb = small_pool.tile([128, 1], FP32)
        scale11_sb = small_pool.tile([128, 1], FP32)
        shift11_sb = small_pool.tile([128, 1], FP32)
        zero_sb = small_pool.tile([128, 1], FP32)
        eps_sb = small_pool.tile([128, 1], FP32)
        nc.any.memset(zero_sb[:], 0.0)
        nc.any.memset(eps_sb[:], BN_EPS)

        for t, p in ((ident_sb, ident_p), (idx_sb, idx_p),
                     (iosl_sb, iosl_p), (imp_sb, imp_p), (ind_sb, ind_p),
                     (w1_sb, w1_p), (w2_sb, w2_p), (b1_sb, b1_p),
                     (b2_sb, b2_p), (ga_sb, ga_p), (be_sb, be_p)):
            nc.sync.dma_start(out=t[:], in_=p[:])

        nc.any.memset(pooled_sb[:], 0.0)
        if NPAD > NL:
            for t in (h_T, h11_T, z_T, hpre_T):
                nc.any.memset(t[:, NL:NPAD], 0.0)

        # h11 for layer 0 = (1+eps) * x_T, streamed
        nch = (NL + CH - 1) // CH
        for j in range(nch):
            lo, hi = j * CH, min((j + 1) * CH, NL)
            xt_c = scr_pool.tile([128, CH], FP32, tag="xt")
            nc.sync.dma_start(out=xt_c[:, :hi - lo], in_=xT_p[:, lo:hi])
            nc.scalar.activation(h11_T[:, lo:hi], xt_c[:, :hi - lo], AF.Copy,
                                 scale=1.0 + EPS_GIN)

        gsrcA, gsrcB = xa_p, xb_p
        nreg_cache = {}

        def nreg(n):
            if n not in nreg_cache:
                nreg_cache[n] = nc.gpsimd.to_reg(n)
            return nreg_cache[n]

        for layer in range(L):
            last_layer = layer == L - 1
            ld = layer * D

            # ---------------- aggregation + MLP ----------------
            # tile offsets per (pass, window) in trace order
            offs = np.zeros((2, W), np.int64)
            o = 0
            for p in range(2):
                for w in range(W):
                    offs[p, w] = o
                    o += int(Tcnt[p, w])

            for p in range(2):
                src = gsrcA if p == 0 else gsrcB
                for g in range(NG):
                    wins = [w for w in (2 * g, 2 * g + 1) if w < W]
                    Tg = sum(int(Tcnt[p, w]) for w in wins)
                    if Tg > 0:
                        off0 = int(offs[p, wins[0]])
                        xb_t = xbuf_pool.tile([128, MAXTG * 128], FP32,
                                              tag="xbuf")
                        nc.gpsimd.dma_gather(
                            xb_t[:, :Tg * 128].rearrange(
                                "p (t f) -> p t f", f=128),
                            src[:, :],
                            idx_sb[:, off0 * 8:(off0 + Tg) * 8],
                            Tg * 128, nreg(Tg * 128), 128, elem_step=128,
                            single_packet=False,
                        )
                    for w in wins:
                        T = int(Tcnt[p, w])
                        ws = w * 128
                        if T > 0:
                            ps = ps_agg.tile([128, 128], FP32, tag="agg")
                            for k in range(T):
                                gt = int(offs[p, w]) + k       # global tile
                                lt = gt - int(offs[p, wins[0]])  # in xbuf
                                s_t = s_pool.tile([128, 128], FP32, tag="s")
                                nc.any.tensor_scalar(
                                    s_t[:], iosl_sb[:, 0:128],
                                    iosl_sb[:, 128 + gt:129 + gt], None,
                                    AX.is_equal)
                                nc.tensor.matmul(
                                    ps[:],
                                    xb_t[:, lt * 128:(lt + 1) * 128],
                                    s_t[:],
                                    start=(k == 0), stop=(k == T - 1))
                        # extract z
                        if p == 0:
                            nc.vector.tensor_add(
                                z_T[:, ws:ws + 128], ps[:],
                                h11_T[:, ws:ws + 128])
                        elif T > 0:
                            nc.vector.tensor_add(
                                z_T[:, ws:ws + 128], z_T[:, ws:ws + 128],
                                ps[:])
                    # MLP for this group after pass 1
                    if p == 1:
                        gs = wins[0] * 128
                        ge = (wins[-1] + 1) * 128
                        gv = min(ge, NL) - gs  # valid cols
                        h1_ps = ps_mlp.tile([128, ge - gs], FP32, tag="mlp")
                        nc.tensor.matmul(h1_ps[:], w1_sb[:, ld:ld + D],
                                         z_T[:, gs:ge], start=True, stop=True)
                        h1_sb = mlp_pool.tile([128, 256], FP32, tag="h1")
                        nc.scalar.activation(h1_sb[:, :ge - gs], h1_ps[:],
                                             AF.Relu,
                                             bias=b1_sb[:, layer:layer + 1])
                        z2_ps = ps_mlp.tile([128, ge - gs], FP32, tag="mlp")
                        nc.tensor.matmul(z2_ps[:], w2_sb[:, ld:ld + D],
                                         h1_sb[:, :ge - gs],
                                         start=True, stop=True)
                        nc.scalar.activation(
                            hpre_T[:, gs:gs + gv], z2_ps[:, :gv], AF.Relu,
                            bias=b2_sb[:, layer:layer + 1],
                            accum_out=sumc_sb[:, g:g + 1])
                        sq_sb = scr_pool.tile([128, 256], FP32, tag="sq")
                        nc.scalar.activation(
                            sq_sb[:, :gv], hpre_T[:, gs:gs + gv], AF.Square,
                            bias=zero_sb[:],
                            accum_out=sqc_sb[:, g:g + 1])

            # ---------------- BN stats AllReduce ----------------
            nc.vector.reduce_sum(stats_sb[:, 0:1], sumc_sb[:], axis=XAX)
            nc.vector.reduce_sum(stats_sb[:, 1:2], sqc_sb[:], axis=XAX)
            bn_in = dram_pool.tile([128, 2], FP32, tag="bn_in")
            bn_out = dram_pool.tile([128, 2], FP32, tag="bn_out",
                                    addr_space="Shared")
            nc.sync.dma_start(out=bn_in[:], in_=stats_sb[:])
            nc.gpsimd.collective_compute(
                "AllReduce", AX.add, replica_groups=RG,
                ins=[bn_in[:].opt()], outs=[bn_out[:].opt()])
            nc.sync.dma_start(out=bn_sb[:], in_=bn_out[:])

            inv_n = 1.0 / float(N)
            nc.scalar.activation(mu_sb[:], bn_sb[:, 0:1], AF.Copy,
                                 scale=inv_n)
            nc.scalar.activation(ex2_sb[:], bn_sb[:, 1:2], AF.Copy,
                                 scale=inv_n)
            nc.vector.tensor_mul(tmp_sb[:], mu_sb[:], mu_sb[:])
            nc.vector.tensor_sub(var_sb[:], ex2_sb[:], tmp_sb[:])
            nc.scalar.activation(std_sb[:], var_sb[:], AF.Sqrt,
                                 bias=eps_sb[:])
            nc.vector.reciprocal(rstd_sb[:], std_sb[:])
            nc.vector.tensor_mul(scale_sb[:], rstd_sb[:],
                                 ga_sb[:, layer:layer + 1])
            nc.vector.tensor_mul(tmp_sb[:], mu_sb[:], scale_sb[:])
            nc.vector.tensor_sub(shift_sb[:], be_sb[:, layer:layer + 1],
                                 tmp_sb[:])
            if not last_layer:
                nc.scalar.activation(scale11_sb[:], scale_sb[:], AF.Copy,
                                     scale=1.0 + EPS_GIN)
                nc.scalar.activation(shift11_sb[:], shift_sb[:], AF.Copy,
                                     scale=1.0 + EPS_GIN)

            # ---------------- BN apply ----------------
            for j in range(nch):
                lo, hi = j * CH, min((j + 1) * CH, NL)
                nc.scalar.activation(h_T[:, lo:hi], hpre_T[:, lo:hi],
                                     AF.Identity, bias=shift_sb[:],
                                     scale=scale_sb[:])
                if not last_layer:
                    nc.scalar.activation(h11_T[:, lo:hi], hpre_T[:, lo:hi],
                                         AF.Identity, bias=shift11_sb[:],
                                         scale=scale11_sb[:])

            # ---------------- output pass: transpose / xs / pool / rows ----
            if not last_layer:
                hs_a = dram_pool.tile([HALF, D], FP32, tag="hsa")
                hs_b = dram_pool.tile([HB, D], FP32, tag="hsb")
            pl_ps = ps_pool.tile([GW, 128], FP32, tag="pool")
            for w in range(W):
                ws = w * 128
                nv = LAST if w == W - 1 else 128
                tr_ps = ps_tr.tile([128, 128], FP32, tag="tr")
                nc.tensor.transpose(tr_ps[:], h_T[:, ws:ws + 128],
                                    ident_sb[:])
                xs_sb = out_pool.tile([128, 128], FP32, tag="xs")
                nc.scalar.activation(xs_sb[:], tr_ps[:], AF.Copy,
                                     scale=imp_sb[:, w:w + 1])
                nc.sync.dma_start(out=xcat_p[ws:ws + nv, ld:ld + D],
                                  in_=xs_sb[:nv, :])
                nc.tensor.matmul(pl_ps[:], ind_sb[:, w * GW:(w + 1) * GW],
                                 xs_sb[:], start=(w == 0), stop=(w == W - 1))
                if not last_layer:
                    rw_sb = out_pool.tile([128, 128], FP32, tag="rw")
                    nc.vector.tensor_copy(rw_sb[:], tr_ps[:])
                    if ws + nv <= HALF:
                        nc.sync.dma_start(out=hs_a[ws:ws + nv, :],
                                          in_=rw_sb[:nv, :])
                    elif ws >= HALF:
                        bs = ws - HALF
                        nc.sync.dma_start(out=hs_b[bs:bs + nv, :],
                                          in_=rw_sb[:nv, :])
                    else:
                        sb = HALF - ws
                        nc.sync.dma_start(out=hs_a[ws:ws + sb, :],
                                          in_=rw_sb[:sb, :])
                        nc.sync.dma_start(out=hs_b[0:nv - sb, :],
                                          in_=rw_sb[sb:nv, :])
            nc.vector.tensor_copy(pooled_sb[:, ld:ld + D], pl_ps[:])

            if not last_layer:
                hA = dram_pool.tile([C * HALF, D], FP32, tag="hA",
                                    addr_space="Shared")
                hB = dram_pool.tile([C * HB, D], FP32, tag="hB",
                                    addr_space="Shared")
                nc.gpsimd.collective_compute(
                    "AllGather", AX.bypass, replica_groups=RG,
                    ins=[hs_a[:].opt()], outs=[hA[:].opt()])
                nc.gpsimd.collective_compute(
                    "AllGather", AX.bypass, replica_groups=RG,
                    ins=[hs_b[:].opt()], outs=[hB[:].opt()])
                gsrcA, gsrcB = hA, hB

        nc.sync.dma_start(out=pool_p[:], in_=pooled_sb[:])

    nc.compile()
    return nc


# ---------------------------------------------------------------------------
# Entry point
# ---------------------------------------------------------------------------

def run(cfg, inputs, run_fn=None):
    """Shared driver: preprocess, build, execute, unshard."""
    meta, shared, per_core = preprocess(
        cfg, inputs["x"], inputs["node_imp"], inputs["edge_index"],
        inputs["batch"])
    wts = pack_weights(cfg, inputs["W1"], inputs["b1"], inputs["W2"],
                       inputs["b2"], inputs["gamma"], inputs["beta"])
    nc = build_program(cfg, meta)

    C = cfg["C"]
    in_maps = []
    for c in range(C):
        m = dict(shared)
        m.update(wts)
        for k, v in per_core.items():
            m[k] = np.ascontiguousarray(v[c])
        in_maps.append(m)

    if run_fn is None:
        from concourse.bass_utils import run_bass_kernel_spmd
        res = run_bass_kernel_spmd(nc, in_maps, list(range(C))).results
    else:
        res = run_fn(nc, in_maps)

    N, G, D, L = cfg["N"], cfg["G"], cfg["D"], cfg["L"]
    dv = derive(cfg)
    NL, GW = dv["NL"], meta["GW"]
    xcat = np.concatenate([res[c]["xcat"] for c in range(C)], axis=0)
    pooled = np.zeros((G, L * D), np.float32)
    for c in range(C):
        g0 = int(meta["gw_start"][c])
        hi = min(g0 + GW, G)
        pooled[g0:hi] += res[c]["pooled"][:hi - g0]
    return pooled, xcat


def kernel(**inputs):
    cfg = full_cfg()
    return run(cfg, inputs)


# revision 23
# speedup vs baseline: 1.1716x; 1.1716x over previous
"""GIN-style 5-layer GNN encoder on 8 TRN2 NeuronCores (Bass/Tile SPMD).

Sharding: nodes split contiguously across 8 cores (6250 each); edges
partitioned by destination core. Per layer, each core gathers the source-node
rows of its incident edges from a replicated row-major copy of h in HBM
(dma_gather), reduces them into 128-node destination windows with 0/1
selection matmuls on the TensorEngine (agg_T[f,slot] += X[e,f].T @ S[e,slot]),
runs the MLP + BatchNorm feat-major, transposes back to node rows on the PE,
and republishes its h shard via a two-phase AllGather (shard halves) that
overlaps the next layer's gather phase. BatchNorm statistics use one small
AllReduce per layer. Graph pooling is a per-window Ind-matmul into a per-core
graph window; per-core partials are combined on the host (unshard step).
"""

import sys

import numpy as np

sys.path.insert(0, "/opt/trn_rl_repo")
sys.path.insert(0, "/opt/trn_rl_repo/concourse")

import concourse.bass as bass
import concourse.mybir as mybir
from concourse import bacc, tile

FP32 = mybir.dt.float32
BF16 = mybir.dt.bfloat16
F32R = mybir.dt.float32r
I16 = mybir.dt.int16
AX = mybir.AluOpType
AF = mybir.ActivationFunctionType
XAX = mybir.AxisListType.X

EPS_GIN = 0.1
BN_EPS = 1e-5
CH = 512  # feat-major chunk width for elementwise passes


def full_cfg():
    return dict(N=50000, E=800000, G=256, D=128, L=5, C=8)


def derive(cfg):
    N, C = cfg["N"], cfg["C"]
    NL = N // C
    assert NL * C == N
    HALF = (NL + 1) // 2
    W = (NL + 127) // 128
    NG = (W + 1) // 2
    return dict(NL=NL, HALF=HALF, HB=NL - HALF, W=W, NG=NG)


# ---------------------------------------------------------------------------
# Host preprocessing (index/layout only + input-vector normalization)
# ---------------------------------------------------------------------------

def preprocess(cfg, x, node_imp, edge_index, batch):
    N, E, G, D, L, C = (cfg[k] for k in ("N", "E", "G", "D", "L", "C"))
    dv = derive(cfg)
    NL, HALF, HB, W, NG = dv["NL"], dv["HALF"], dv["HB"], dv["W"], dv["NG"]

    x = np.ascontiguousarray(np.asarray(x, np.float32))
    imp_raw = np.asarray(node_imp, np.float32)[:, 0]
    src = np.asarray(edge_index[0], np.int64)
    dst = np.asarray(edge_index[1], np.int64)
    batch = np.asarray(batch, np.int64)

    gmax = np.full(G, -np.inf, np.float32)
    np.maximum.at(gmax, batch, imp_raw)
    gmax = np.where(np.isfinite(gmax), gmax, np.float32(1.0)).astype(np.float32)
    imp = (imp_raw / (gmax[batch] * 10.0) + 0.9).astype(np.float32)

    owner = src // NL
    pos = src % NL
    in_a = pos < HALF
    srow = np.where(in_a, owner * HALF + pos,
                    owner * HB + (pos - HALF)).astype(np.int64)
    ehalf = (~in_a).astype(np.int64)

    ecore = dst // NL
    ldst = dst % NL
    ewin = ldst // 128
    eslot = ldst % 128

    per_core = []
    for c in range(C):
        m = ecore == c
        order = np.lexsort((srow[m], ewin[m], ehalf[m]))
        per_core.append((ehalf[m][order], ewin[m][order],
                         srow[m][order], eslot[m][order]))

    # tiles per (pass, window), equal across cores
    Tcnt = np.zeros((2, W), np.int64)
    for c in range(C):
        h_, w_, _, _ = per_core[c]
        for p in range(2):
            for w in range(W):
                n = int(np.sum((h_ == p) & (w_ == w)))
                Tcnt[p, w] = max(Tcnt[p, w], (n + 127) // 128)
    Tcnt[0] = np.maximum(Tcnt[0], 1)  # every window gets >=1 pass-0 tile
    TOT_TILES = int(Tcnt.sum())
    TOT_IDX = TOT_TILES * 128

    idx_flat = np.zeros((C, TOT_IDX), np.int16)
    slot_np = np.full((C, 128, TOT_TILES), 255.0, np.float32)
    for c in range(C):
        h_, w_, r_, s_ = per_core[c]
        off = 0
        for p in range(2):
            for w in range(W):
                T = int(Tcnt[p, w])
                if T == 0:
                    continue
                sel = (h_ == p) & (w_ == w)
                rows, slots = r_[sel], s_[sel]
                n = len(rows)
                ii = np.zeros(T * 128, np.int64)
                ii[:n] = rows
                ss = np.full(T * 128, 255.0, np.float32)
                ss[:n] = slots
                idx_flat[c, off * 128:(off + T) * 128] = ii.astype(np.int16)
                slot_np[c, :, off:off + T] = ss.reshape(T, 128).T
                off += T
        assert off == TOT_TILES

    idx_wrap = np.zeros((C, 128, TOT_IDX // 16), np.int16)
    for c in range(C):
        idx_wrap[c] = np.tile(idx_flat[c].reshape(-1, 16).T, (8, 1))

    xa = np.zeros((C * HALF, D), np.float32)
    xb = np.zeros((C * HB, D), np.float32)
    for c in range(C):
        xs_ = x[c * NL:(c + 1) * NL]
        xa[c * HALF:(c + 1) * HALF] = xs_[:HALF]
        xb[c * HB:(c + 1) * HB] = xs_[HALF:]

    NPAD = W * 128
    xT = np.zeros((C, D, NPAD), np.float32)
    imp_loc = np.zeros((C, 128, W), np.float32)
    for c in range(C):
        xT[c, :, :NL] = x[c * NL:(c + 1) * NL].T
        v = np.zeros(NPAD, np.float32)
        v[:NL] = imp[c * NL:(c + 1) * NL]
        imp_loc[c] = v.reshape(W, 128).T

    GW = min(64, G)
    gw_start = np.zeros(C, np.int64)
    ind_np = np.zeros((C, 128, W * GW), np.float32)
    for c in range(C):
        gb = batch[c * NL:(c + 1) * NL]
        g0 = int(gb.min())
        gw_start[c] = g0
        assert int(gb.max()) - g0 < GW, "graph window overflow"
        cols = (gb - g0).astype(np.int64)
        for n in range(NL):
            ind_np[c, n % 128, (n // 128) * GW + cols[n]] = 1.0

    meta = dict(Tcnt=Tcnt, TOT_TILES=TOT_TILES, TOT_IDX=TOT_IDX, GW=GW,
                gw_start=gw_start, NPAD=NPAD)
    iota = np.tile(np.arange(128, dtype=np.float32), (128, 1))
    ioslot = np.concatenate(
        [np.broadcast_to(iota[None], (C, 128, 128)), slot_np], axis=2)
    per_core_inputs = dict(xT=xT, idx=idx_wrap, ioslot=ioslot, imp=imp_loc,
                           ind=ind_np)
    shared_inputs = dict(
        xa=xa, xb=xb,
        ident=np.eye(128, dtype=np.float32),
    )
    return meta, shared_inputs, per_core_inputs


def pack_weights(cfg, W1, b1, W2, b2, gamma, beta):
    D, L = cfg["D"], cfg["L"]
    w1s = np.zeros((D, L * D), np.float32)
    w2s = np.zeros((D, L * D), np.float32)
    for l in range(L):
        w1s[:, l * D:(l + 1) * D] = W1[l]
        w2s[:, l * D:(l + 1) * D] = W2[l]
    return dict(
        w1s=w1s, w2s=w2s,
        b1s=np.asarray(b1, np.float32).T.copy(),
        b2s=np.asarray(b2, np.float32).T.copy(),
        gammas=np.asarray(gamma, np.float32).T.copy(),
        betas=np.asarray(beta, np.float32).T.copy(),
    )


# ---------------------------------------------------------------------------
# Device program (identical across cores)
# ---------------------------------------------------------------------------

def build_program(cfg, meta):
    N, E, G, D, L, C = (cfg[k] for k in ("N", "E", "G", "D", "L", "C"))
    dv = derive(cfg)
    NL, HALF, HB, W, NG = dv["NL"], dv["HALF"], dv["HB"], dv["W"], dv["NG"]
    Tcnt, TOT_TILES, TOT_IDX = meta["Tcnt"], meta["TOT_TILES"], meta["TOT_IDX"]
    GW, NPAD = meta["GW"], meta["NPAD"]
    RG = [list(range(C))]
    LAST = NL - (W - 1) * 128       # valid rows in the last window
    WB = HALF // 128                # window straddling the A/B shard boundary
    SB = HALF % 128

    nc = bacc.Bacc("TRN2", target_bir_lowering=False, debug=False,
                   num_devices=C)

    def param(name, shape, dtype=FP32, out=False):
        return nc.declare_dram_parameter(name, list(shape), dtype,
                                         isOutput=out)

    xa_p = param("xa", [C * HALF, D])
    xb_p = param("xb", [C * HB, D])
    xT_p = param("xT", [D, NPAD])
    idx_p = param("idx", [128, TOT_IDX // 16], I16)
    iosl_p = param("ioslot", [128, 128 + TOT_TILES])
    imp_p = param("imp", [128, W])
    ind_p = param("ind", [128, W * GW])
    ident_p = param("ident", [128, 128])
    w1_p = param("w1s", [D, L * D])
    w2_p = param("w2s", [D, L * D])
    b1_p = param("b1s", [D, L])
    b2_p = param("b2s", [D, L])
    ga_p = param("gammas", [D, L])
    be_p = param("betas", [D, L])

    xcat_p = param("xcat", [NL, L * D], out=True)
    pool_p = param("pooled", [GW, L * D], out=True)

    MAXTG = max(int(Tcnt[p, 2 * g] + (Tcnt[p, 2 * g + 1] if 2 * g + 1 < W
                                      else 0))
                for p in range(2) for g in range(NG))

    from contextlib import ExitStack

    with tile.TileContext(nc) as tc, ExitStack() as ctx:
        const_pool = ctx.enter_context(tc.tile_pool(name="consts", bufs=1))
        big_pool = ctx.enter_context(tc.tile_pool(name="big", bufs=1))
        xbuf_pool = ctx.enter_context(tc.tile_pool(name="xbuf", bufs=2))
        s_pool = ctx.enter_context(tc.tile_pool(name="spool", bufs=4))
        small_pool = ctx.enter_context(tc.tile_pool(name="small", bufs=1))
        mlp_pool = ctx.enter_context(tc.tile_pool(name="mlp", bufs=2))
        scr_pool = ctx.enter_context(tc.tile_pool(name="scr", bufs=2))
        out_pool = ctx.enter_context(tc.tile_pool(name="outp", bufs=3))
        ps_agg = ctx.enter_context(
            tc.tile_pool(name="ps_agg", bufs=3, space="PSUM"))
        ps_mlp = ctx.enter_context(
            tc.tile_pool(name="ps_mlp", bufs=2, space="PSUM"))
        ps_tr = ctx.enter_context(
            tc.tile_pool(name="ps_tr", bufs=2, space="PSUM"))
        ps_pool = ctx.enter_context(
            tc.tile_pool(name="ps_pool", bufs=1, space="PSUM"))
        dram_pool = ctx.enter_context(
            tc.tile_pool(name="dram", bufs=2, space="DRAM"))

        # persistent SBUF
        ident_sb = const_pool.tile([128, 128], FP32)
        idx_sb = const_pool.tile([128, TOT_IDX // 16], I16)
        iosl_sb = const_pool.tile([128, 128 + TOT_TILES], FP32)
        imp_sb = const_pool.tile([128, W], FP32)
        ind_sb = const_pool.tile([128, W * GW], FP32)
        w1_sb = const_pool.tile([D, L * D], FP32)
        w2_sb = const_pool.tile([D, L * D], FP32)
        b1_sb = const_pool.tile([D, L], FP32)
        b2_sb = const_pool.tile([D, L], FP32)
        ga_sb = const_pool.tile([D, L], FP32)
        be_sb = const_pool.tile([D, L], FP32)

        h_T = big_pool.tile([D, NPAD], FP32)
        h11_T = big_pool.tile([D, NPAD], FP32)
        z_T = big_pool.tile([D, NPAD], FP32)
        hpre_T = big_pool.tile([D, NPAD], FP32)
        pooled_sb = big_pool.tile([GW, L * D], FP32)
        sumc_sb = small_pool.tile([128, NG], FP32)
        sqc_sb = small_pool.tile([128, NG], FP32)
        stats_sb = small_pool.tile([128, 2], FP32)
        bn_sb = small_pool.tile([128, 2], FP32)
        mu_sb = small_pool.tile([128, 1], FP32)
        ex2_sb = small_pool.tile([128, 1], FP32)
        var_sb = small_pool.tile([128, 1], FP32)
        std_sb = small_pool.tile([128, 1], FP32)
        rstd_sb = small_pool.tile([128, 1], FP32)
        scale_sb = small_pool.tile([128, 1], FP32)
        shift_sb = small_pool.tile([128, 1], FP32)
        tmp_sb = small_pool.tile([128, 1], FP32)
        scale11_sb = small_pool.tile([128, 1], FP32)
        shift11_sb = small_pool.tile([128, 1], FP32)
        zero_sb = small_pool.tile([128, 1], FP32)
        eps_sb = small_pool.tile([128, 1], FP32)
        nc.any.memset(zero_sb[:], 0.0)
        nc.any.memset(eps_sb[:], BN_EPS)

        for t, p in ((ident_sb, ident_p), (idx_sb, idx_p),
                     (iosl_sb, iosl_p), (imp_sb, imp_p), (ind_sb, ind_p),
                     (w1_sb, w1_p), (w2_sb, w2_p), (b1_sb, b1_p),
                     (b2_sb, b2_p), (ga_sb, ga_p), (be_sb, be_p)):
            nc.sync.dma_start(out=t[:], in_=p[:])

        nc.any.memset(pooled_sb[:], 0.0)
        if NPAD > NL:
            for t in (h_T, h11_T, z_T, hpre_T):
                nc.any.memset(t[:, NL:NPAD], 0.0)

        # h11 for layer 0 = (1+eps) * x_T, streamed
        nch = (NL + CH - 1) // CH
        for j in range(nch):
            lo, hi = j * CH, min((j + 1) * CH, NL)
            xt_c = scr_pool.tile([128, CH], FP32, tag="xt")
            nc.sync.dma_start(out=xt_c[:, :hi - lo], in_=xT_p[:, lo:hi])
            nc.scalar.activation(h11_T[:, lo:hi], xt_c[:, :hi - lo], AF.Copy,
                                 scale=1.0 + EPS_GIN)

        gsrcA, gsrcB = xa_p, xb_p
        nreg_cache = {}

        def nreg(n):
            if n not in nreg_cache:
                nreg_cache[n] = nc.gpsimd.to_reg(n)
            return nreg_cache[n]

        for layer in range(L):
            last_layer = layer == L - 1
            ld = layer * D

            # ---------------- aggregation + MLP ----------------
            # tile offsets per (pass, window) in trace order
            offs = np.zeros((2, W), np.int64)
            o = 0
            for p in range(2):
                for w in range(W):
                    offs[p, w] = o
                    o += int(Tcnt[p, w])

            GCH = 8  # tiles per dma_gather (<=1024 idx, single-packet safe)
            for p in range(2):
                src = gsrcA if p == 0 else gsrcB
                for g in range(NG):
                    wins = [w for w in (2 * g, 2 * g + 1) if w < W]
                    Tg = sum(int(Tcnt[p, w]) for w in wins)
                    if Tg > 0:
                        off0 = int(offs[p, wins[0]])
                        xb_t = xbuf_pool.tile([128, MAXTG * 128], F32R,
                                              tag="xbuf")
                        for c0 in range(0, Tg, GCH):
                            cn = min(GCH, Tg - c0)
                            nc.gpsimd.dma_gather(
                                xb_t[:, c0 * 128:(c0 + cn) * 128].rearrange(
                                    "p (t f) -> p t f", f=128),
                                src[:, :].bitcast(F32R),
                                idx_sb[:, (off0 + c0) * 8:
                                       (off0 + c0 + cn) * 8],
                                cn * 128, nreg(cn * 128), 128, elem_step=128,
                            )
                        # batched S build: one broadcast is_equal per group
                        s_t = s_pool.tile([128, MAXTG * 128], F32R, tag="s")
                        iota3 = (iosl_sb[:, 0:128].unsqueeze(1)
                                 .broadcast_to([128, Tg, 128]))
                        slot3 = (iosl_sb[:, 128 + off0:128 + off0 + Tg]
                                 .unsqueeze(2).broadcast_to([128, Tg, 128]))
                        nc.vector.tensor_tensor(
                            s_t[:, :Tg * 128].rearrange(
                                "p (t f) -> p t f", f=128),
                            iota3, slot3, AX.is_equal)
                    for w in wins:
                        T = int(Tcnt[p, w])
                        ws = w * 128
                        if T > 0:
                            ps = ps_agg.tile([128, 128], FP32, tag="agg")
                            for k in range(T):
                                lt = int(offs[p, w]) + k - int(
                                    offs[p, wins[0]])  # tile idx in group
                                nc.tensor.matmul(
                                    ps[:],
                                    xb_t[:, lt * 128:(lt + 1) * 128],
                                    s_t[:, lt * 128:(lt + 1) * 128],
                                    start=(k == 0), stop=(k == T - 1))
                        # extract z
                        if p == 0:
                            nc.vector.tensor_add(
                                z_T[:, ws:ws + 128], ps[:],
                                h11_T[:, ws:ws + 128])
                        elif T > 0:
                            nc.vector.tensor_add(
                                z_T[:, ws:ws + 128], z_T[:, ws:ws + 128],
                                ps[:])
                    # MLP for this group after pass 1
                    if p == 1:
                        gs = wins[0] * 128
                        ge = (wins[-1] + 1) * 128
                        gv = min(ge, NL) - gs  # valid cols
                        h1_ps = ps_mlp.tile([128, ge - gs], FP32, tag="mlp")
                        nc.tensor.matmul(h1_ps[:], w1_sb[:, ld:ld + D],
                                         z_T[:, gs:ge], start=True, stop=True)
                        h1_sb = mlp_pool.tile([128, 256], FP32, tag="h1")
                        nc.scalar.activation(h1_sb[:, :ge - gs], h1_ps[:],
                                             AF.Relu,
                                             bias=b1_sb[:, layer:layer + 1])
                        z2_ps = ps_mlp.tile([128, ge - gs], FP32, tag="mlp")
                        nc.tensor.matmul(z2_ps[:], w2_sb[:, ld:ld + D],
                                         h1_sb[:, :ge - gs],
                                         start=True, stop=True)
                        nc.scalar.activation(
                            hpre_T[:, gs:gs + gv], z2_ps[:, :gv], AF.Relu,
                            bias=b2_sb[:, layer:layer + 1],
                            accum_out=sumc_sb[:, g:g + 1])
                        sq_sb = scr_pool.tile([128, 256], FP32, tag="sq")
                        nc.scalar.activation(
                            sq_sb[:, :gv], hpre_T[:, gs:gs + gv], AF.Square,
                            bias=zero_sb[:],
                            accum_out=sqc_sb[:, g:g + 1])

            # ---------------- BN stats AllReduce ----------------
            nc.vector.reduce_sum(stats_sb[:, 0:1], sumc_sb[:], axis=XAX)
            nc.vector.reduce_sum(stats_sb[:, 1:2], sqc_sb[:], axis=XAX)
            bn_in = dram_pool.tile([128, 2], FP32, tag="bn_in")
            bn_out = dram_pool.tile([128, 2], FP32, tag="bn_out",
                                    addr_space="Shared")
            nc.sync.dma_start(out=bn_in[:], in_=stats_sb[:])
            nc.gpsimd.collective_compute(
                "AllReduce", AX.add, replica_groups=RG,
                ins=[bn_in[:].opt()], outs=[bn_out[:].opt()])
            nc.sync.dma_start(out=bn_sb[:], in_=bn_out[:])

            inv_n = 1.0 / float(N)
            nc.scalar.activation(mu_sb[:], bn_sb[:, 0:1], AF.Copy,
                                 scale=inv_n)
            nc.scalar.activation(ex2_sb[:], bn_sb[:, 1:2], AF.Copy,
                                 scale=inv_n)
            nc.vector.tensor_mul(tmp_sb[:], mu_sb[:], mu_sb[:])
            nc.vector.tensor_sub(var_sb[:], ex2_sb[:], tmp_sb[:])
            nc.scalar.activation(std_sb[:], var_sb[:], AF.Sqrt,
                                 bias=eps_sb[:])
            nc.vector.reciprocal(rstd_sb[:], std_sb[:])
            nc.vector.tensor_mul(scale_sb[:], rstd_sb[:],
                                 ga_sb[:, layer:layer + 1])
            nc.vector.tensor_mul(tmp_sb[:], mu_sb[:], scale_sb[:])
            nc.vector.tensor_sub(shift_sb[:], be_sb[:, layer:layer + 1],
                                 tmp_sb[:])
            if not last_layer:
                nc.scalar.activation(scale11_sb[:], scale_sb[:], AF.Copy,
                                     scale=1.0 + EPS_GIN)
                nc.scalar.activation(shift11_sb[:], shift_sb[:], AF.Copy,
                                     scale=1.0 + EPS_GIN)

            # ---------------- BN apply ----------------
            for j in range(nch):
                lo, hi = j * CH, min((j + 1) * CH, NL)
                nc.scalar.activation(h_T[:, lo:hi], hpre_T[:, lo:hi],
                                     AF.Identity, bias=shift_sb[:],
                                     scale=scale_sb[:])
                if not last_layer:
                    nc.scalar.activation(h11_T[:, lo:hi], hpre_T[:, lo:hi],
                                         AF.Identity, bias=shift11_sb[:],
                                         scale=scale11_sb[:])

            # ---------------- output pass: transpose / xs / pool / rows ----
            if not last_layer:
                hs_a = dram_pool.tile([HALF, D], FP32, tag="hsa")
                hs_b = dram_pool.tile([HB, D], FP32, tag="hsb")
            pl_ps = ps_pool.tile([GW, 128], FP32, tag="pool")
            for w in range(W):
                ws = w * 128
                nv = LAST if w == W - 1 else 128
                tr_ps = ps_tr.tile([128, 128], FP32, tag="tr")
                nc.tensor.transpose(tr_ps[:], h_T[:, ws:ws + 128],
                                    ident_sb[:])
                xs_sb = out_pool.tile([128, 128], FP32, tag="xs")
                nc.scalar.activation(xs_sb[:], tr_ps[:], AF.Copy,
                                     scale=imp_sb[:, w:w + 1])
                nc.sync.dma_start(out=xcat_p[ws:ws + nv, ld:ld + D],
                                  in_=xs_sb[:nv, :])
                nc.tensor.matmul(pl_ps[:], ind_sb[:, w * GW:(w + 1) * GW],
                                 xs_sb[:], start=(w == 0), stop=(w == W - 1))
                if not last_layer:
                    rw_sb = out_pool.tile([128, 128], FP32, tag="rw")
                    nc.vector.tensor_copy(rw_sb[:], tr_ps[:])
                    if ws + nv <= HALF:
                        nc.sync.dma_start(out=hs_a[ws:ws + nv, :],
                                          in_=rw_sb[:nv, :])
                    elif ws >= HALF:
                        bs = ws - HALF
                        nc.sync.dma_start(out=hs_b[bs:bs + nv, :],
                                          in_=rw_sb[:nv, :])
                    else:
                        sb = HALF - ws
                        nc.sync.dma_start(out=hs_a[ws:ws + sb, :],
                                          in_=rw_sb[:sb, :])
                        nc.sync.dma_start(out=hs_b[0:nv - sb, :],
                                          in_=rw_sb[sb:nv, :])
            nc.vector.tensor_copy(pooled_sb[:, ld:ld + D], pl_ps[:])

            if not last_layer:
                hA = dram_pool.tile([C * HALF, D], FP32, tag="hA",
                                    addr_space="Shared")
                hB = dram_pool.tile([C * HB, D], FP32, tag="hB",
                                    addr_space="Shared")
                nc.gpsimd.collective_compute(
                    "AllGather", AX.bypass, replica_groups=RG,
                    ins=[hs_a[:].opt()], outs=[hA[:].opt()])
                nc.gpsimd.collective_compute(
                    "AllGather", AX.bypass, replica_groups=RG,
                    ins=[hs_b[:].opt()], outs=[hB[:].opt()])
                gsrcA, gsrcB = hA, hB

        nc.sync.dma_start(out=pool_p[:], in_=pooled_sb[:])

    nc.compile()
    return nc


# ---------------------------------------------------------------------------
# Entry point
# ---------------------------------------------------------------------------

def run(cfg, inputs, run_fn=None):
    """Shared driver: preprocess, build, execute, unshard."""
    meta, shared, per_core = preprocess(
        cfg, inputs["x"], inputs["node_imp"], inputs["edge_index"],
        inputs["batch"])
    wts = pack_weights(cfg, inputs["W1"], inputs["b1"], inputs["W2"],
                       inputs["b2"], inputs["gamma"], inputs["beta"])
    nc = build_program(cfg, meta)

    C = cfg["C"]
    in_maps = []
    for c in range(C):
        m = dict(shared)
        m.update(wts)
        for k, v in per_core.items():
            m[k] = np.ascontiguousarray(v[c])
        in_maps.append(m)

    if run_fn is None:
        from concourse.bass_utils import run_bass_kernel_spmd
        res = run_bass_kernel_spmd(nc, in_maps, list(range(C))).results
    else:
        res = run_fn(nc, in_maps)

    N, G, D, L = cfg["N"], cfg["G"], cfg["D"], cfg["L"]
    dv = derive(cfg)
    NL, GW = dv["NL"], meta["GW"]
    xcat = np.concatenate([res[c]["xcat"] for c in range(C)], axis=0)
    pooled = np.zeros((G, L * D), np.float32)
    for c in range(C):
        g0 = int(meta["gw_start"][c])
        hi = min(g0 + GW, G)
        pooled[g0:hi] += res[c]["pooled"][:hi - g0]
    return pooled, xcat


def kernel(**inputs):
    cfg = full_cfg()
    return run(cfg, inputs)
